# revision 42
# baseline (speedup 1.0000x reference)
"""Multi-head attention (B=2, S=2048, H=1024, 16 heads) on 8 TRN2 NeuronCores.

Sharding: core c -> batch b = c//4, head-group g = c%4 (heads 4g..4g+3).
Each core computes q/k/v projections for its 4 heads (tensor parallel),
full attention for those heads, and a partial output projection
(contribution of its 256 hidden dims). Host sums the 4 partials per batch
and adds the output bias.

Changes over the first working kernel (268.8us -> ~221us):
  * software-pipelined attention inner loop: PV(k-2) is emitted two
    iterations behind scores(k), so the PE never waits on the ACT exp
    (the v1 loop serialized scores->exp->PV every jj, ~0.5us/iter stall).
  * merged scores stationary: one [128,128] kt chunk serves both heads of
    a pair; the per-head q operand is zero-padded on the other 64
    partitions ("qt_pad"), so the second LDWEIGHTS is redundant.
  * post-compile LDWEIGHTS dedupe: bass emits one LDWEIGHTS per matmul
    (~100ns of PE time each, 848 total = 85us in v1); a module pass
    removes loads whose weights AP + tile config match the immediately
    preceding load on the PE queue (only matmuls/event-sems between, no
    waits/updates on the removed load).
  * emission order maximizes consecutive same-stationary matmuls
    (qk-proj streams s-blocks per weight chunk, out-proj streams both
    e-blocks per ctxn chunk, bias/bc pairs share their stationary).
  * "big" input tiles ([128, 8, free]) loaded with ONE DMA per logical
    block via a (e p) c -> p e c rearrange: per-DMA dispatch is ~0.6us
    of queue time, so the old 26-DMA startup serialized ~15us of input;
    the pair-0 k/q weight halves go on the scalar-engine DGE queue, the
    bulk on the sync queue, cutting first-exp from ~37us to ~23us.
  * pair-1 projections and v-projections hooked INTO attention(0) so the
    exp pipeline never drains between head-pairs; partial output
    projection stores bf16 (host sums in fp32).
  * block-tail carry: each block's last two PVs + epilogue are emitted in
    the NEXT block's first two iterations, and the deferred normalize
    (early, k==4) / out-proj (boundary) split keeps the ctxn shift DMA
    off the out-proj critical path - the PE no longer parks ~1-2us at
    each of the 8 block boundaries.

Rejected experiments (measured): fp8 DoubleRow matmuls (4x PV / 2x proj
throughput) fail the 2e-2 accuracy gate (e4m3 ~2.6%/elem); custom-DVE
exp/reciprocal tables don't load under this runtime; out-DMA from PSUM
is unsupported; scalar-queue DMA dispatch during the exp phase delays
ACT; boundary-filler proj hooks serialize on the mip psum pool.
"""

import os
from contextlib import ExitStack

import numpy as np
import ml_dtypes

B = 2
S = 2048
HID = 1024
NHEAD = 16
HDIM = 64
NCORES = 8
GROUPS = 4  # head-groups per batch (cores per batch)
DH = 256  # hidden dims per core (4 heads x 64)
SCALE = 1.0 / np.sqrt(np.float32(HDIM))  # 0.125

_CACHE = {}
last_exec_time_ns = None
last_results = None


def _dedupe_ldweights(nc):
    """Remove InstLdweights that reload the PE array with the exact weights
    the previous load already placed there. bass always emits one LDWEIGHTS
    per matmul; when consecutive matmuls share lhsT the reload is ~100ns of
    pure PE overhead. Only loads with no waits/updates are removed, and the
    match window is broken by anything on the PE queue other than matmuls
    and event-semaphore instructions (and at block boundaries)."""
    import concourse.mybir as mybir

    removed = 0
    for fn in nc.m.functions:
        for bb in fn.blocks:
            insts = list(bb.instructions)
            last_sig = None
            keep = []
            for inst in insts:
                if inst.engine != mybir.EngineType.PE:
                    keep.append(inst)
                    continue
                tname = type(inst).__name__
                if tname == "InstLdweights":
                    sig = (
                        str(inst.ins[0]),
                        str(inst.perf_mode),
                        str(inst.is_transpose),
                        str(inst.tile_position),
                        str(inst.tile_size),
                    )
                    if (
                        sig == last_sig
                        and not inst.has_wait()
                        and not inst.has_update()
                    ):
                        removed += 1
                        continue  # drop it
                    last_sig = sig
                    keep.append(inst)
                elif tname == "InstMatmult":
                    keep.append(inst)
                elif tname == "InstEventSemaphore":
                    keep.append(inst)
                else:
                    # drain/branch/anything else: invalidate the window
                    last_sig = None
                    keep.append(inst)
            if removed:
                bb.instructions = keep
    return removed


def _build_graph(with_qkv_bias: bool):
    import concourse.bass as bass
    import concourse.mybir as mybir
    import concourse.tile as tile
    from concourse import bacc

    F32 = mybir.dt.float32
    BF16 = mybir.dt.bfloat16
    EXP = mybir.ActivationFunctionType.Exp
    LN = mybir.ActivationFunctionType.Ln

    # Keep the activation-table steering from v1: with it, the act-table
    # load pass settles on one table set (1 load) instead of 17.
    if not getattr(bacc, "_mha_act_tabs_patched", False):
        orig_gat = bacc.get_activation_tables

        def _gat(arch, _orig=orig_gat):
            out = {}
            for n, s in _orig(arch).items():
                if n != "natural_log_exp_and_others":
                    s = s - {EXP, LN}
                out[n] = s
            return out

        bacc.get_activation_tables = _gat
        bacc._mha_act_tabs_patched = True

    nc = bacc.Bacc()
    xt_d = nc.declare_dram_parameter("xt", [HID, S], BF16, isOutput=False)
    wq_d = nc.declare_dram_parameter("wq", [HID, DH], BF16, isOutput=False)
    wk_d = nc.declare_dram_parameter("wk", [HID, DH], BF16, isOutput=False)
    wv_d = nc.declare_dram_parameter("wv", [HID, DH], BF16, isOutput=False)
    wo_d = nc.declare_dram_parameter("wo", [DH, HID], BF16, isOutput=False)
    if with_qkv_bias:
        bq_d = nc.declare_dram_parameter("bq", [1, DH], BF16, isOutput=False)
        bk_d = nc.declare_dram_parameter("bk", [1, DH], BF16, isOutput=False)
        bv_d = nc.declare_dram_parameter("bv", [1, DH], BF16, isOutput=False)
    out_d = nc.declare_dram_parameter("out", [S, HID], BF16, isOutput=True)

    with ExitStack() as ctx:
        tc = ctx.enter_context(tile.TileContext(nc))
        cons = ctx.enter_context(tc.tile_pool(name="cons", bufs=1))
        work = ctx.enter_context(tc.tile_pool(name="work", bufs=3))
        scp = ctx.enter_context(tc.tile_pool(name="scp", bufs=2, space="PSUM"))
        pvp = ctx.enter_context(tc.tile_pool(name="pvp", bufs=1, space="PSUM"))
        mip = ctx.enter_context(tc.tile_pool(name="mip", bufs=2, space="PSUM"))

        # ---- persistent SBUF tiles ------------------------------------
        # "big" tiles pack all 8 hidden chunks: [128, 8, free]. One DMA
        # moves a whole logical block (dispatch cost was serializing the
        # startup at ~0.6us per DMA on the queues).
        xt_big = cons.tile([128, 8, S], BF16, name="xt", tag="xt")
        wq_big = cons.tile([128, 8, DH], BF16, name="wq", tag="wq")
        wk_big = cons.tile([128, 8, DH], BF16, name="wk", tag="wk")
        wv_big = cons.tile([128, 8, DH], BF16, name="wv", tag="wv")
        xt_sb = [xt_big[:, e, :] for e in range(8)]
        wq_sb = [wq_big[:, e, :] for e in range(8)]
        wk_sb = [wk_big[:, e, :] for e in range(8)]
        wv_sb = [wv_big[:, e, :] for e in range(8)]
        wo_sb = [
            cons.tile([128, HID], BF16, name=f"wo{e}", tag=f"wo{e}") for e in range(2)
        ]

        def load_big(eng, out_ap, dram, cols):
            eng.dma_start(
                out=out_ap,
                in_=dram[:, cols].rearrange("(e p) c -> p e c", e=8),
            )

        # scalar queue: the small pair-0 weight halves that gate the first
        # k/q chains, then the last xt block.  sync queue: everything else
        # in consumption order.
        load_big(nc.scalar, wk_big[:, :, 0:128], wk_d, slice(0, 128))
        load_big(nc.scalar, wq_big[:, :, 0:128], wq_d, slice(0, 128))
        load_big(nc.scalar, wv_big, wv_d, slice(0, DH))
        load_big(nc.scalar, xt_big[:, :, 1536:2048], xt_d, slice(1536, 2048))
        # sb0 in two e-halves so the first projection chain starts earlier
        for half in range(2):
            rows = slice(half * 512, (half + 1) * 512)
            nc.sync.dma_start(
                out=xt_big[:, half * 4 : (half + 1) * 4, 0:512],
                in_=xt_d[rows, 0:512].rearrange("(e p) c -> p e c", e=4),
            )
        for half in range(2):
            rows = slice(half * 512, (half + 1) * 512)
            nc.sync.dma_start(
                out=xt_big[:, half * 4 : (half + 1) * 4, 512:1024],
                in_=xt_d[rows, 512:1024].rearrange("(e p) c -> p e c", e=4),
            )
        load_big(nc.sync, xt_big[:, :, 1024:1536], xt_d, slice(1024, 1536))
        load_big(nc.sync, wk_big[:, :, 128:256], wk_d, slice(128, 256))
        load_big(nc.sync, wq_big[:, :, 128:256], wq_d, slice(128, 256))
        for e in range(2):
            nc.sync.dma_start(out=wo_sb[e], in_=wo_d[e * 128 : (e + 1) * 128, :])

        ones1 = cons.tile([1, 512], BF16, name="ones1", tag="ones1")
        nc.vector.memset(ones1, 1.0)
        # stationary of the K=1 broadcast matmul for 1/l (partition base 0)
        ones1b = cons.tile([1, 64], BF16, name="ones1b", tag="ones1b")
        nc.vector.memset(ones1b, 1.0)

        if with_qkv_bias:
            bias_sb = {}
            for nm, d in (("bq", bq_d), ("bk", bk_d), ("bv", bv_d)):
                t = cons.tile([1, DH], BF16, name=f"{nm}s", tag=f"{nm}s")
                nc.sync.dma_start(out=t, in_=d)
                bias_sb[nm] = t

        # qt zero-padded per (pair, h): data rows h*64..h*64+63, rest 0.
        qt_pad = [
            [
                cons.tile([128, S], BF16, name=f"qp{c}{h}", tag=f"qp{c}{h}")
                for h in range(2)
            ]
            for c in range(2)
        ]
        # zero them once (proj only overwrites the data half); gpsimd for
        # half of them to spread the startup memset cost.
        nc.vector.memset(qt_pad[0][0][64:128, :], 0.0)
        nc.vector.memset(qt_pad[0][1][0:64, :], 0.0)
        nc.gpsimd.memset(qt_pad[1][0][64:128, :], 0.0)
        nc.gpsimd.memset(qt_pad[1][1][0:64, :], 0.0)

        kt_sb = [
            cons.tile([128, S], BF16, name=f"kt{c}", tag=f"kt{c}") for c in range(2)
        ]
        v_sb = [
            cons.tile([128, 4, 65], BF16, name=f"v{j}", tag=f"v{j}") for j in range(16)
        ]
        ctxn_sb = [
            [
                cons.tile([128, 512], BF16, name=f"cx{c}_{i}", tag=f"cx{c}_{i}")
                for i in range(4)
            ]
            for c in range(2)
        ]

        # ---- projections ----------------------------------------------
        def proj_qk(which, pair, sbs):
            """q or k projection for one pair over the given s-blocks,
            streaming all of them per weight chunk so consecutive matmuls
            share their stationary (dedupable LDWEIGHTS)."""
            w_sb = wq_sb if which == "q" else wk_sb
            ps = [
                mip.tile([128, 512], F32, name=f"p{which}{pair}{sb}", tag="mm")
                for sb in sbs
            ]
            for e in range(8):
                lhs = w_sb[e][:, pair * 128 : (pair + 1) * 128]
                for i, sb in enumerate(sbs):
                    nc.tensor.matmul(
                        ps[i],
                        lhsT=lhs,
                        rhs=xt_sb[e][:, sb * 512 : (sb + 1) * 512],
                        start=(e == 0),
                        stop=(e == 7 and not with_qkv_bias),
                    )
            if with_qkv_bias:
                bias = bias_sb["bq" if which == "q" else "bk"]
                lhs = bias[:, pair * 128 : (pair + 1) * 128]
                for i in range(len(sbs)):
                    nc.tensor.matmul(ps[i], lhsT=lhs, rhs=ones1, start=False, stop=True)
            for i, sb in enumerate(sbs):
                sl = slice(sb * 512, (sb + 1) * 512)
                if which == "k":
                    nc.vector.tensor_copy(out=kt_sb[pair][:, sl], in_=ps[i])
                else:
                    nc.vector.tensor_copy(
                        out=qt_pad[pair][0][0:64, sl], in_=ps[i][0:64, :]
                    )
                    nc.vector.tensor_copy(
                        out=qt_pad[pair][1][64:128, sl], in_=ps[i][64:128, :]
                    )

        def proj_v_one(jj):
            # v [s, d] natural, per j-chunk [128, 4, 65] with ones at 64.
            ps = mip.tile([128, DH], F32, name=f"pv{jj}", tag="mm")
            for e in range(8):
                nc.tensor.matmul(
                    ps,
                    lhsT=xt_sb[e][:, jj * 128 : (jj + 1) * 128],
                    rhs=wv_sb[e],
                    start=(e == 0),
                    stop=(e == 7 and not with_qkv_bias),
                )
            if with_qkv_bias:
                nc.tensor.matmul(
                    ps,
                    lhsT=ones1[:, 0:128],
                    rhs=bias_sb["bv"],
                    start=False,
                    stop=True,
                )
            nc.vector.tensor_copy(
                out=v_sb[jj][:, :, 0:64],
                in_=ps.rearrange("p (h d) -> p h d", h=4),
            )
            nc.vector.memset(v_sb[jj][:, :, 64:65], 1.0)

        # ---- deferred epilogue work ------------------------------------
        # part2 (bc matmuls + normalize + ctxn shift DMA) flushes EARLY in
        # the following block so the shift DMA is long done when the
        # out-projection (flushed at that block's END) reads ctxn rows
        # 64-127 — otherwise the PE parks ~1us on the DMA at the boundary.
        deferred_early = []
        deferred_late = []

        def flush_deferred(lst):
            while lst:
                lst.pop(0)()

        def outproj(ib):
            # partial output projection, e-block-streamed per ctxn chunk,
            # PSUM DMA'd straight to DRAM
            for ss in range(4):
                po = [
                    mip.tile([128, 512], F32, name=f"po{ib}{ss}{i}", tag="mm")
                    for i in range(2)
                ]
                for cc in range(2):
                    lhs = ctxn_sb[cc][ib][:, ss * 128 : (ss + 1) * 128]
                    for eb in range(2):
                        nc.tensor.matmul(
                            po[eb],
                            lhsT=lhs,
                            rhs=wo_sb[cc][:, eb * 512 : (eb + 1) * 512],
                            start=(cc == 0),
                            stop=(cc == 1),
                        )
                row = ib * 512 + ss * 128
                for eb in range(2):
                    ot = work.tile(
                        [128, 512], BF16, name=f"ot{ib}{ss}{eb}", tag="ot", bufs=4
                    )
                    nc.vector.tensor_copy(out=ot, in_=po[eb])
                    # the final block's stores race the kernel end; spread
                    # them over both DGE queues (ACT is idle by then)
                    eng = nc.scalar if (ib == 3 and eb == 1) else nc.sync
                    eng.dma_start(
                        out=out_d[row : row + 128, eb * 512 : (eb + 1) * 512],
                        in_=ot,
                    )

        # ---- attention -------------------------------------------------
        # Each block's last two PVs (which wait on the freshest exps) and
        # its epilogue are carried into the NEXT block's first iterations,
        # so the PE never parks at a block boundary.
        pending = []

        def epilogue(pair, ib, pv):
            # psum->sbuf copies free the pv banks; 1/l as exp(-ln(l)) on
            # ACT (custom-DVE reciprocal tables don't load in this
            # environment's compiler path).
            pvs_l, rl16_l = [], []
            for h in range(2):
                pvs = work.tile(
                    [65, 512], F32, name=f"pvs{pair}{ib}{h}", tag="pvs", bufs=6
                )
                nc.vector.tensor_copy(out=pvs, in_=pv[h])
                lnl = work.tile(
                    [1, 512], F32, name=f"lnl{pair}{ib}{h}", tag="lnl", bufs=2
                )
                nc.scalar.activation(out=lnl, in_=pvs[64:65, :], func=LN)
                rl16 = work.tile(
                    [1, 512], BF16, name=f"rl16{pair}{ib}{h}", tag="rl16", bufs=6
                )
                nc.scalar.activation(out=rl16, in_=lnl, func=EXP, scale=-1.0)
                pvs_l.append(pvs)
                rl16_l.append(rl16)

            def part2(pair=pair, ib=ib, pvs_l=pvs_l, rl16_l=rl16_l):
                bc_t = []
                for h in range(2):
                    bc = mip.tile([64, 512], F32, name=f"bc{pair}{ib}{h}", tag="mm")
                    nc.tensor.matmul(
                        bc, lhsT=ones1b, rhs=rl16_l[h], start=True, stop=True
                    )
                    bc_t.append(bc)
                # DVE may read only one PSUM operand: in0 SBUF, in1 PSUM.
                nc.vector.tensor_mul(
                    out=ctxn_sb[pair][ib][0:64, :],
                    in0=pvs_l[0][0:64, :],
                    in1=bc_t[0],
                )
                tmp = work.tile([64, 512], BF16, name=f"tmp{pair}{ib}", tag="tmp")
                nc.vector.tensor_mul(out=tmp, in0=pvs_l[1][0:64, :], in1=bc_t[1])
                nc.sync.dma_start(out=ctxn_sb[pair][ib][64:128, :], in_=tmp)

            deferred_early.append(part2)
            if pair == 1:
                deferred_late.append(lambda ib=ib: outproj(ib))

        def attention(pair, hooks=None, last=False):
            hooks = hooks or {}
            for ib in range(4):
                pv = [
                    pvp.tile([65, 512], F32, name=f"pva{pair}{ib}", tag="pva"),
                    pvp.tile([65, 512], F32, name=f"pvb{pair}{ib}", tag="pvb"),
                ]
                ex_t = [None] * 16

                def emit_pv(jj, pair=pair, pv=pv, ex_t=ex_t):
                    for h in range(2):
                        nc.tensor.matmul(
                            pv[h],
                            lhsT=v_sb[jj][:, pair * 2 + h, :],
                            rhs=ex_t[jj][:, h * 512 : (h + 1) * 512],
                            start=(jj == 0),
                            stop=(jj == 15),
                        )

                for k in list(range(16)) + [16]:
                    for fn in hooks.get((ib, k), ()):
                        fn()
                    if k == 16:
                        break
                    ps = scp.tile([128, 1024], F32, name=f"sc{pair}{ib}{k}", tag="sc")
                    ktsl = kt_sb[pair][:, k * 128 : (k + 1) * 128]
                    for h in range(2):
                        nc.tensor.matmul(
                            ps[:, h * 512 : (h + 1) * 512],
                            lhsT=ktsl,
                            rhs=qt_pad[pair][h][:, ib * 512 : (ib + 1) * 512],
                            start=True,
                            stop=True,
                        )
                    ex = work.tile(
                        [128, 1024], BF16, name=f"ex{pair}{ib}{k}", tag="ex", bufs=4
                    )
                    nc.scalar.activation(out=ex, in_=ps, func=EXP, scale=float(SCALE))
                    ex_t[k] = ex
                    if k == 0 and pending:
                        pending[0]()
                    elif k == 1 and pending:
                        pending[1]()
                        pending.clear()
                    elif k == 4:
                        flush_deferred(deferred_early)
                    if k >= 2:
                        emit_pv(k - 2)

                # out-proj of the PREVIOUS block lands at the boundary (its
                # ctxn was finished early in this block); the last block
                # leaves it for the final tail so the fresh PVs go first.
                if not (last and ib == 3):
                    flush_deferred(deferred_late)
                pending.append(lambda e=emit_pv: e(14))
                pending.append(
                    lambda e=emit_pv, p=pair, i=ib, t=pv: (e(15), epilogue(p, i, t))
                )

        # ---- emission order --------------------------------------------
        # minimal prelude: the first scores chunk only needs kt sb0 and
        # qt sb0, each an 8-matmul chain off the first DMA wave.
        proj_qk("k", 0, [0])
        proj_qk("q", 0, [0])

        hooks0 = {}
        # remaining pair-0 projections, j-chunk v projections, and ALL of
        # pair-1's projections are hooked into attention(0) so the exp
        # pipeline never drains.  DMA-gated work is hooked just-in-time
        # (v[jj] right before its PV at iteration jj+2) — an early hook
        # whose input hasn't landed blocks the statically-ordered PE
        # queue head and stalls everything behind it.
        for jj in range(14):
            hooks0[(0, jj + 2)] = [lambda jj=jj: proj_v_one(jj)]
        hooks0[(0, 16)] = [lambda: proj_v_one(14), lambda: proj_v_one(15)]
        hooks0[(0, 3)].append(lambda: proj_qk("k", 0, [1]))
        hooks0[(0, 6)].append(lambda: proj_qk("k", 0, [2, 3]))
        hooks0[(0, 10)].append(lambda: proj_qk("q", 0, [1]))
        hooks0[(1, 2)] = [lambda: proj_qk("q", 0, [2, 3])]
        hooks0[(2, 2)] = [lambda: proj_qk("k", 1, [0, 1])]
        hooks0[(2, 9)] = [lambda: proj_qk("k", 1, [2, 3])]
        hooks0[(3, 2)] = [lambda: proj_qk("q", 1, [0, 1])]
        hooks0[(3, 9)] = [lambda: proj_qk("q", 1, [2, 3])]

        attention(0, hooks0)
        attention(1, last=True)
        # final tail: the last block's PVs + epilogue go FIRST (their exps
        # are the freshest dependency); the held-back out-proj of block 6
        # fills the epilogue's ACT latency; then normalize + out-proj 7.
        for fn in pending:
            fn()
        pending.clear()
        if deferred_late:
            deferred_late.pop(0)()  # out-proj of the second-to-last block
        flush_deferred(deferred_early)
        flush_deferred(deferred_late)

    nc.compile()
    n = _dedupe_ldweights(nc)
    nc._mha_deduped_ldweights = n
    return nc


def _get_graph(with_qkv_bias: bool):
    key = ("nc", with_qkv_bias)
    if key not in _CACHE:
        _CACHE[key] = _build_graph(with_qkv_bias)
    return _CACHE[key]


def make_in_maps(x, Wq, bq, Wk, bk, Wv, bv, Wo, with_qkv_bias):
    bf16 = ml_dtypes.bfloat16
    in_maps = []
    for c in range(NCORES):
        b, g = c // GROUPS, c % GROUPS
        hs = slice(g * DH, (g + 1) * DH)
        m = {
            "xt": np.ascontiguousarray(x[b].T.astype(bf16)),
            "wq": np.ascontiguousarray(Wq[hs, :].T.astype(bf16)),
            "wk": np.ascontiguousarray(Wk[hs, :].T.astype(bf16)),
            "wv": np.ascontiguousarray(Wv[hs, :].T.astype(bf16)),
            "wo": np.ascontiguousarray(Wo[:, hs].T.astype(bf16)),
        }
        if with_qkv_bias:
            m["bq"] = np.ascontiguousarray(bq[None, hs].astype(bf16))
            m["bk"] = np.ascontiguousarray(bk[None, hs].astype(bf16))
            m["bv"] = np.ascontiguousarray(bv[None, hs].astype(bf16))
        in_maps.append(m)
    return in_maps


def kernel(x, Wq, bq, Wk, bk, Wv, bv, Wo, bo):
    global last_exec_time_ns, last_results
    from concourse.bass_utils import run_bass_kernel_spmd

    x = np.asarray(x, np.float32)
    Wq = np.asarray(Wq, np.float32)
    Wk = np.asarray(Wk, np.float32)
    Wv = np.asarray(Wv, np.float32)
    Wo = np.asarray(Wo, np.float32)
    bq = np.asarray(bq, np.float32)
    bk = np.asarray(bk, np.float32)
    bv = np.asarray(bv, np.float32)
    bo = np.asarray(bo, np.float32)

    with_qkv_bias = bool(np.any(bq) or np.any(bk) or np.any(bv))
    nc = _get_graph(with_qkv_bias)
    in_maps = make_in_maps(x, Wq, bq, Wk, bk, Wv, bv, Wo, with_qkv_bias)

    trace = os.environ.get("BASS_KERNEL_TRACE", "0") == "1"
    tdir = os.environ.get("BASS_KERNEL_TRACE_DIR") or None
    res = run_bass_kernel_spmd(
        nc, in_maps, list(range(NCORES)), trace=trace, tmpdir=tdir
    )
    last_exec_time_ns = res.exec_time_ns
    last_results = res

    out = np.zeros((B, S, HID), np.float32)
    for c in range(NCORES):
        out[c // GROUPS] += res.results[c]["out"]
    out += bo
    return out


# revision 43
# speedup vs baseline: 1.0176x; 1.0176x over previous
"""Multi-head attention (B=2, S=2048, H=1024, 16 heads) on 8 TRN2 NeuronCores.

Sharding: core c -> batch b = c//4, head-group g = c%4 (heads 4g..4g+3).
Each core computes q/k/v projections for its 4 heads (tensor parallel),
full attention for those heads, and a partial output projection
(contribution of its 256 hidden dims). Host sums the 4 partials per batch
and adds the output bias.

Changes over the first working kernel (268.8us -> ~221us):
  * software-pipelined attention inner loop: PV(k-2) is emitted two
    iterations behind scores(k), so the PE never waits on the ACT exp
    (the v1 loop serialized scores->exp->PV every jj, ~0.5us/iter stall).
  * merged scores stationary: one [128,128] kt chunk serves both heads of
    a pair; the per-head q operand is zero-padded on the other 64
    partitions ("qt_pad"), so the second LDWEIGHTS is redundant.
  * post-compile LDWEIGHTS dedupe: bass emits one LDWEIGHTS per matmul
    (~100ns of PE time each, 848 total = 85us in v1); a module pass
    removes loads whose weights AP + tile config match the immediately
    preceding load on the PE queue (only matmuls/event-sems between, no
    waits/updates on the removed load).
  * emission order maximizes consecutive same-stationary matmuls
    (qk-proj streams s-blocks per weight chunk, out-proj streams both
    e-blocks per ctxn chunk, bias/bc pairs share their stationary).
  * "big" input tiles ([128, 8, free]) loaded with ONE DMA per logical
    block via a (e p) c -> p e c rearrange: per-DMA dispatch is ~0.6us
    of queue time, so the old 26-DMA startup serialized ~15us of input;
    the pair-0 k/q weight halves go on the scalar-engine DGE queue, the
    bulk on the sync queue, cutting first-exp from ~37us to ~23us.
  * pair-1 projections and v-projections hooked INTO attention(0) so the
    exp pipeline never drains between head-pairs; partial output
    projection stores bf16 (host sums in fp32).
  * block-tail carry: each block's last two PVs + epilogue are emitted in
    the NEXT block's first two iterations, and the deferred normalize
    (early, k==4) / out-proj (boundary) split keeps the ctxn shift DMA
    off the out-proj critical path - the PE no longer parks ~1-2us at
    each of the 8 block boundaries.

Rejected experiments (measured): fp8 DoubleRow matmuls (4x PV / 2x proj
throughput) fail the 2e-2 accuracy gate (e4m3 ~2.6%/elem); custom-DVE
exp/reciprocal tables don't load under this runtime; out-DMA from PSUM
is unsupported; scalar-queue DMA dispatch during the exp phase delays
ACT; boundary-filler proj hooks serialize on the mip psum pool.
"""

import os
from contextlib import ExitStack

import numpy as np
import ml_dtypes

B = 2
S = 2048
HID = 1024
NHEAD = 16
HDIM = 64
NCORES = 8
GROUPS = 4  # head-groups per batch (cores per batch)
DH = 256  # hidden dims per core (4 heads x 64)
SCALE = 1.0 / np.sqrt(np.float32(HDIM))  # 0.125

_CACHE = {}
last_exec_time_ns = None
last_results = None


def _dedupe_ldweights(nc):
    """Remove InstLdweights that reload the PE array with the exact weights
    the previous load already placed there. bass always emits one LDWEIGHTS
    per matmul; when consecutive matmuls share lhsT the reload is ~100ns of
    pure PE overhead. Only loads with no waits/updates are removed, and the
    match window is broken by anything on the PE queue other than matmuls
    and event-semaphore instructions (and at block boundaries)."""
    import concourse.mybir as mybir

    removed = 0
    for fn in nc.m.functions:
        for bb in fn.blocks:
            insts = list(bb.instructions)
            last_sig = None
            keep = []
            for inst in insts:
                if inst.engine != mybir.EngineType.PE:
                    keep.append(inst)
                    continue
                tname = type(inst).__name__
                if tname == "InstLdweights":
                    sig = (
                        str(inst.ins[0]),
                        str(inst.perf_mode),
                        str(inst.is_transpose),
                        str(inst.tile_position),
                        str(inst.tile_size),
                    )
                    if (
                        sig == last_sig
                        and not inst.has_wait()
                        and not inst.has_update()
                    ):
                        removed += 1
                        continue  # drop it
                    last_sig = sig
                    keep.append(inst)
                elif tname == "InstMatmult":
                    keep.append(inst)
                elif tname == "InstEventSemaphore":
                    keep.append(inst)
                else:
                    # drain/branch/anything else: invalidate the window
                    last_sig = None
                    keep.append(inst)
            if removed:
                bb.instructions = keep
    return removed


def _build_graph(with_qkv_bias: bool):
    import concourse.bass as bass
    import concourse.mybir as mybir
    import concourse.tile as tile
    from concourse import bacc

    F32 = mybir.dt.float32
    BF16 = mybir.dt.bfloat16
    EXP = mybir.ActivationFunctionType.Exp
    LN = mybir.ActivationFunctionType.Ln

    # Keep the activation-table steering from v1: with it, the act-table
    # load pass settles on one table set (1 load) instead of 17.
    if not getattr(bacc, "_mha_act_tabs_patched", False):
        orig_gat = bacc.get_activation_tables

        def _gat(arch, _orig=orig_gat):
            out = {}
            for n, s in _orig(arch).items():
                if n != "natural_log_exp_and_others":
                    s = s - {EXP, LN}
                out[n] = s
            return out

        bacc.get_activation_tables = _gat
        bacc._mha_act_tabs_patched = True

    nc = bacc.Bacc()
    xt_d = nc.declare_dram_parameter("xt", [HID, S], BF16, isOutput=False)
    wq_d = nc.declare_dram_parameter("wq", [HID, DH], BF16, isOutput=False)
    wk_d = nc.declare_dram_parameter("wk", [HID, DH], BF16, isOutput=False)
    wv_d = nc.declare_dram_parameter("wv", [HID, DH], BF16, isOutput=False)
    wo_d = nc.declare_dram_parameter("wo", [DH, HID], BF16, isOutput=False)
    if with_qkv_bias:
        bq_d = nc.declare_dram_parameter("bq", [1, DH], BF16, isOutput=False)
        bk_d = nc.declare_dram_parameter("bk", [1, DH], BF16, isOutput=False)
        bv_d = nc.declare_dram_parameter("bv", [1, DH], BF16, isOutput=False)
    out_d = nc.declare_dram_parameter("out", [S, HID], BF16, isOutput=True)

    with ExitStack() as ctx:
        tc = ctx.enter_context(tile.TileContext(nc))
        cons = ctx.enter_context(tc.tile_pool(name="cons", bufs=1))
        work = ctx.enter_context(tc.tile_pool(name="work", bufs=3))
        scp = ctx.enter_context(tc.tile_pool(name="scp", bufs=2, space="PSUM"))
        pvp = ctx.enter_context(tc.tile_pool(name="pvp", bufs=1, space="PSUM"))
        mip = ctx.enter_context(tc.tile_pool(name="mip", bufs=2, space="PSUM"))

        # ---- persistent SBUF tiles ------------------------------------
        # "big" tiles pack all 8 hidden chunks: [128, 8, free]. One DMA
        # moves a whole logical block (dispatch cost was serializing the
        # startup at ~0.6us per DMA on the queues).
        xt_big = cons.tile([128, 8, S], BF16, name="xt", tag="xt")
        wq_big = cons.tile([128, 8, DH], BF16, name="wq", tag="wq")
        wk_big = cons.tile([128, 8, DH], BF16, name="wk", tag="wk")
        wv_big = cons.tile([128, 8, DH], BF16, name="wv", tag="wv")
        xt_sb = [xt_big[:, e, :] for e in range(8)]
        wq_sb = [wq_big[:, e, :] for e in range(8)]
        wk_sb = [wk_big[:, e, :] for e in range(8)]
        wv_sb = [wv_big[:, e, :] for e in range(8)]
        wo_sb = [
            cons.tile([128, HID], BF16, name=f"wo{e}", tag=f"wo{e}") for e in range(2)
        ]

        def load_big(eng, out_ap, dram, cols):
            eng.dma_start(
                out=out_ap,
                in_=dram[:, cols].rearrange("(e p) c -> p e c", e=8),
            )

        # scalar queue: the small pair-0 weight halves that gate the first
        # k/q chains, then the last xt block.  sync queue: everything else
        # in consumption order.
        # scalar queue: pair-0 k/q weight halves (gate the first chains),
        # then xt sb1 and the pair-1 halves.  sync queue: xt sb0 halves,
        # wv early (v-projections start at ~iteration 0), then the rest.
        load_big(nc.scalar, wk_big[:, :, 0:128], wk_d, slice(0, 128))
        load_big(nc.scalar, wq_big[:, :, 0:128], wq_d, slice(0, 128))
        for half in range(2):
            rows = slice(half * 512, (half + 1) * 512)
            nc.scalar.dma_start(
                out=xt_big[:, half * 4 : (half + 1) * 4, 512:1024],
                in_=xt_d[rows, 512:1024].rearrange("(e p) c -> p e c", e=4),
            )
        load_big(nc.scalar, wk_big[:, :, 128:256], wk_d, slice(128, 256))
        load_big(nc.scalar, wq_big[:, :, 128:256], wq_d, slice(128, 256))
        for half in range(2):
            rows = slice(half * 512, (half + 1) * 512)
            nc.sync.dma_start(
                out=xt_big[:, half * 4 : (half + 1) * 4, 0:512],
                in_=xt_d[rows, 0:512].rearrange("(e p) c -> p e c", e=4),
            )
        load_big(nc.sync, wv_big, wv_d, slice(0, DH))
        load_big(nc.sync, xt_big[:, :, 1024:1536], xt_d, slice(1024, 1536))
        load_big(nc.sync, xt_big[:, :, 1536:2048], xt_d, slice(1536, 2048))
        for e in range(2):
            nc.sync.dma_start(out=wo_sb[e], in_=wo_d[e * 128 : (e + 1) * 128, :])

        ones1 = cons.tile([1, 512], BF16, name="ones1", tag="ones1")
        nc.vector.memset(ones1, 1.0)
        # stationary of the K=1 broadcast matmul for 1/l (partition base 0)
        ones1b = cons.tile([1, 64], BF16, name="ones1b", tag="ones1b")
        nc.vector.memset(ones1b, 1.0)

        if with_qkv_bias:
            bias_sb = {}
            for nm, d in (("bq", bq_d), ("bk", bk_d), ("bv", bv_d)):
                t = cons.tile([1, DH], BF16, name=f"{nm}s", tag=f"{nm}s")
                nc.sync.dma_start(out=t, in_=d)
                bias_sb[nm] = t

        # qt zero-padded per (pair, h): data rows h*64..h*64+63, rest 0.
        qt_pad = [
            [
                cons.tile([128, S], BF16, name=f"qp{c}{h}", tag=f"qp{c}{h}")
                for h in range(2)
            ]
            for c in range(2)
        ]
        # zero them once (proj only overwrites the data half); gpsimd for
        # half of them to spread the startup memset cost.
        nc.vector.memset(qt_pad[0][0][64:128, :], 0.0)
        nc.vector.memset(qt_pad[0][1][0:64, :], 0.0)
        nc.gpsimd.memset(qt_pad[1][0][64:128, :], 0.0)
        nc.gpsimd.memset(qt_pad[1][1][0:64, :], 0.0)

        kt_sb = [
            cons.tile([128, S], BF16, name=f"kt{c}", tag=f"kt{c}") for c in range(2)
        ]
        v_sb = [
            cons.tile([128, 4, 65], BF16, name=f"v{j}", tag=f"v{j}") for j in range(16)
        ]
        ctxn_sb = [
            [
                cons.tile([128, 512], BF16, name=f"cx{c}_{i}", tag=f"cx{c}_{i}")
                for i in range(4)
            ]
            for c in range(2)
        ]

        # ---- projections ----------------------------------------------
        def proj_qk(which, pair, sbs):
            """q or k projection for one pair over the given s-blocks,
            streaming all of them per weight chunk so consecutive matmuls
            share their stationary (dedupable LDWEIGHTS)."""
            w_sb = wq_sb if which == "q" else wk_sb
            ps = [
                mip.tile([128, 512], F32, name=f"p{which}{pair}{sb}", tag="mm")
                for sb in sbs
            ]
            for e in range(8):
                lhs = w_sb[e][:, pair * 128 : (pair + 1) * 128]
                for i, sb in enumerate(sbs):
                    nc.tensor.matmul(
                        ps[i],
                        lhsT=lhs,
                        rhs=xt_sb[e][:, sb * 512 : (sb + 1) * 512],
                        start=(e == 0),
                        stop=(e == 7 and not with_qkv_bias),
                    )
            if with_qkv_bias:
                bias = bias_sb["bq" if which == "q" else "bk"]
                lhs = bias[:, pair * 128 : (pair + 1) * 128]
                for i in range(len(sbs)):
                    nc.tensor.matmul(ps[i], lhsT=lhs, rhs=ones1, start=False, stop=True)
            for i, sb in enumerate(sbs):
                sl = slice(sb * 512, (sb + 1) * 512)
                if which == "k":
                    nc.vector.tensor_copy(out=kt_sb[pair][:, sl], in_=ps[i])
                else:
                    nc.vector.tensor_copy(
                        out=qt_pad[pair][0][0:64, sl], in_=ps[i][0:64, :]
                    )
                    nc.vector.tensor_copy(
                        out=qt_pad[pair][1][64:128, sl], in_=ps[i][64:128, :]
                    )

        def proj_v_one(jj):
            # v [s, d] natural, per j-chunk [128, 4, 65] with ones at 64.
            ps = mip.tile([128, DH], F32, name=f"pv{jj}", tag="mm")
            for e in range(8):
                nc.tensor.matmul(
                    ps,
                    lhsT=xt_sb[e][:, jj * 128 : (jj + 1) * 128],
                    rhs=wv_sb[e],
                    start=(e == 0),
                    stop=(e == 7 and not with_qkv_bias),
                )
            if with_qkv_bias:
                nc.tensor.matmul(
                    ps,
                    lhsT=ones1[:, 0:128],
                    rhs=bias_sb["bv"],
                    start=False,
                    stop=True,
                )
            nc.vector.tensor_copy(
                out=v_sb[jj][:, :, 0:64],
                in_=ps.rearrange("p (h d) -> p h d", h=4),
            )
            nc.vector.memset(v_sb[jj][:, :, 64:65], 1.0)

        # ---- deferred epilogue work ------------------------------------
        # part2 (bc matmuls + normalize + ctxn shift DMA) flushes EARLY in
        # the following block so the shift DMA is long done when the
        # out-projection (flushed at that block's END) reads ctxn rows
        # 64-127 — otherwise the PE parks ~1us on the DMA at the boundary.
        deferred_early = []
        deferred_late = []

        def flush_deferred(lst):
            while lst:
                lst.pop(0)()

        def outproj(ib):
            # partial output projection, e-block-streamed per ctxn chunk,
            # PSUM DMA'd straight to DRAM
            for ss in range(4):
                po = [
                    mip.tile([128, 512], F32, name=f"po{ib}{ss}{i}", tag="mm")
                    for i in range(2)
                ]
                for cc in range(2):
                    lhs = ctxn_sb[cc][ib][:, ss * 128 : (ss + 1) * 128]
                    for eb in range(2):
                        nc.tensor.matmul(
                            po[eb],
                            lhsT=lhs,
                            rhs=wo_sb[cc][:, eb * 512 : (eb + 1) * 512],
                            start=(cc == 0),
                            stop=(cc == 1),
                        )
                row = ib * 512 + ss * 128
                for eb in range(2):
                    ot = work.tile(
                        [128, 512], BF16, name=f"ot{ib}{ss}{eb}", tag="ot", bufs=4
                    )
                    nc.vector.tensor_copy(out=ot, in_=po[eb])
                    # the final block's stores race the kernel end; spread
                    # them over both DGE queues (ACT is idle by then)
                    eng = nc.scalar if (ib == 3 and eb == 1) else nc.sync
                    eng.dma_start(
                        out=out_d[row : row + 128, eb * 512 : (eb + 1) * 512],
                        in_=ot,
                    )

        # ---- attention -------------------------------------------------
        # Each block's last two PVs (which wait on the freshest exps) and
        # its epilogue are carried into the NEXT block's first iterations,
        # so the PE never parks at a block boundary.
        pending = []

        def epilogue(pair, ib, pv):
            # psum->sbuf copies free the pv banks; 1/l as exp(-ln(l)) on
            # ACT (custom-DVE reciprocal tables don't load in this
            # environment's compiler path).
            pvs_l, rl16_l = [], []
            for h in range(2):
                pvs = work.tile(
                    [65, 512], F32, name=f"pvs{pair}{ib}{h}", tag="pvs", bufs=6
                )
                nc.vector.tensor_copy(out=pvs, in_=pv[h])
                lnl = work.tile(
                    [1, 512], F32, name=f"lnl{pair}{ib}{h}", tag="lnl", bufs=2
                )
                nc.scalar.activation(out=lnl, in_=pvs[64:65, :], func=LN)
                rl16 = work.tile(
                    [1, 512], BF16, name=f"rl16{pair}{ib}{h}", tag="rl16", bufs=6
                )
                nc.scalar.activation(out=rl16, in_=lnl, func=EXP, scale=-1.0)
                pvs_l.append(pvs)
                rl16_l.append(rl16)

            def part2(pair=pair, ib=ib, pvs_l=pvs_l, rl16_l=rl16_l):
                bc_t = []
                for h in range(2):
                    bc = mip.tile([64, 512], F32, name=f"bc{pair}{ib}{h}", tag="mm")
                    nc.tensor.matmul(
                        bc, lhsT=ones1b, rhs=rl16_l[h], start=True, stop=True
                    )
                    bc_t.append(bc)
                # DVE may read only one PSUM operand: in0 SBUF, in1 PSUM.
                nc.vector.tensor_mul(
                    out=ctxn_sb[pair][ib][0:64, :],
                    in0=pvs_l[0][0:64, :],
                    in1=bc_t[0],
                )
                tmp = work.tile([64, 512], BF16, name=f"tmp{pair}{ib}", tag="tmp")
                nc.vector.tensor_mul(out=tmp, in0=pvs_l[1][0:64, :], in1=bc_t[1])
                nc.sync.dma_start(out=ctxn_sb[pair][ib][64:128, :], in_=tmp)

            deferred_early.append(part2)
            if pair == 1:
                deferred_late.append(lambda ib=ib: outproj(ib))

        def attention(pair, hooks=None, last=False):
            hooks = hooks or {}
            for ib in range(4):
                pv = [
                    pvp.tile([65, 512], F32, name=f"pva{pair}{ib}", tag="pva"),
                    pvp.tile([65, 512], F32, name=f"pvb{pair}{ib}", tag="pvb"),
                ]
                ex_t = [None] * 16

                def emit_pv(jj, pair=pair, pv=pv, ex_t=ex_t):
                    for h in range(2):
                        nc.tensor.matmul(
                            pv[h],
                            lhsT=v_sb[jj][:, pair * 2 + h, :],
                            rhs=ex_t[jj][:, h * 512 : (h + 1) * 512],
                            start=(jj == 0),
                            stop=(jj == 15),
                        )

                for k in list(range(16)) + [16]:
                    for fn in hooks.get((ib, k), ()):
                        fn()
                    if k == 16:
                        break
                    ps = scp.tile([128, 1024], F32, name=f"sc{pair}{ib}{k}", tag="sc")
                    ktsl = kt_sb[pair][:, k * 128 : (k + 1) * 128]
                    for h in range(2):
                        nc.tensor.matmul(
                            ps[:, h * 512 : (h + 1) * 512],
                            lhsT=ktsl,
                            rhs=qt_pad[pair][h][:, ib * 512 : (ib + 1) * 512],
                            start=True,
                            stop=True,
                        )
                    ex = work.tile(
                        [128, 1024], BF16, name=f"ex{pair}{ib}{k}", tag="ex", bufs=4
                    )
                    nc.scalar.activation(out=ex, in_=ps, func=EXP, scale=float(SCALE))
                    ex_t[k] = ex
                    if k == 0 and pending:
                        pending[0]()
                    elif k == 1 and pending:
                        pending[1]()
                        pending.clear()
                    elif k == 4:
                        flush_deferred(deferred_early)
                    if k >= 2:
                        emit_pv(k - 2)

                # out-proj of the PREVIOUS block lands at the boundary (its
                # ctxn was finished early in this block); the last block
                # leaves it for the final tail so the fresh PVs go first.
                if not (last and ib == 3):
                    flush_deferred(deferred_late)
                pending.append(lambda e=emit_pv: e(14))
                pending.append(
                    lambda e=emit_pv, p=pair, i=ib, t=pv: (e(15), epilogue(p, i, t))
                )

        # ---- emission order --------------------------------------------
        # minimal prelude: the first scores chunk only needs kt sb0 and
        # qt sb0, each an 8-matmul chain off the first DMA wave.
        proj_qk("k", 0, [0])
        proj_qk("q", 0, [0])

        hooks0 = {}
        # remaining pair-0 projections, j-chunk v projections, and ALL of
        # pair-1's projections are hooked into attention(0) so the exp
        # pipeline never drains.  DMA-gated work is hooked just-in-time
        # (v[jj] right before its PV at iteration jj+2) — an early hook
        # whose input hasn't landed blocks the statically-ordered PE
        # queue head and stalls everything behind it.
        for jj in range(14):
            hooks0[(0, jj + 2)] = [lambda jj=jj: proj_v_one(jj)]
        hooks0[(0, 16)] = [lambda: proj_v_one(14), lambda: proj_v_one(15)]
        hooks0[(0, 3)].append(lambda: proj_qk("k", 0, [1]))
        hooks0[(0, 6)].append(lambda: proj_qk("k", 0, [2, 3]))
        hooks0[(0, 10)].append(lambda: proj_qk("q", 0, [1]))
        hooks0[(1, 2)] = [lambda: proj_qk("q", 0, [2, 3])]
        hooks0[(2, 2)] = [lambda: proj_qk("k", 1, [0, 1])]
        hooks0[(2, 9)] = [lambda: proj_qk("k", 1, [2, 3])]
        hooks0[(3, 2)] = [lambda: proj_qk("q", 1, [0, 1])]
        hooks0[(3, 9)] = [lambda: proj_qk("q", 1, [2, 3])]

        attention(0, hooks0)
        attention(1, last=True)
        # final tail: the last block's PVs + epilogue go FIRST (their exps
        # are the freshest dependency); the held-back out-proj of block 6
        # fills the epilogue's ACT latency; then normalize + out-proj 7.
        for fn in pending:
            fn()
        pending.clear()
        if deferred_late:
            deferred_late.pop(0)()  # out-proj of the second-to-last block
        flush_deferred(deferred_early)
        flush_deferred(deferred_late)

    nc.compile()
    n = _dedupe_ldweights(nc)
    nc._mha_deduped_ldweights = n
    return nc


def _get_graph(with_qkv_bias: bool):
    key = ("nc", with_qkv_bias)
    if key not in _CACHE:
        _CACHE[key] = _build_graph(with_qkv_bias)
    return _CACHE[key]


def make_in_maps(x, Wq, bq, Wk, bk, Wv, bv, Wo, with_qkv_bias):
    bf16 = ml_dtypes.bfloat16
    in_maps = []
    for c in range(NCORES):
        b, g = c // GROUPS, c % GROUPS
        hs = slice(g * DH, (g + 1) * DH)
        m = {
            "xt": np.ascontiguousarray(x[b].T.astype(bf16)),
            "wq": np.ascontiguousarray(Wq[hs, :].T.astype(bf16)),
            "wk": np.ascontiguousarray(Wk[hs, :].T.astype(bf16)),
            "wv": np.ascontiguousarray(Wv[hs, :].T.astype(bf16)),
            "wo": np.ascontiguousarray(Wo[:, hs].T.astype(bf16)),
        }
        if with_qkv_bias:
            m["bq"] = np.ascontiguousarray(bq[None, hs].astype(bf16))
            m["bk"] = np.ascontiguousarray(bk[None, hs].astype(bf16))
            m["bv"] = np.ascontiguousarray(bv[None, hs].astype(bf16))
        in_maps.append(m)
    return in_maps


def kernel(x, Wq, bq, Wk, bk, Wv, bv, Wo, bo):
    global last_exec_time_ns, last_results
    from concourse.bass_utils import run_bass_kernel_spmd

    x = np.asarray(x, np.float32)
    Wq = np.asarray(Wq, np.float32)
    Wk = np.asarray(Wk, np.float32)
    Wv = np.asarray(Wv, np.float32)
    Wo = np.asarray(Wo, np.float32)
    bq = np.asarray(bq, np.float32)
    bk = np.asarray(bk, np.float32)
    bv = np.asarray(bv, np.float32)
    bo = np.asarray(bo, np.float32)

    with_qkv_bias = bool(np.any(bq) or np.any(bk) or np.any(bv))
    nc = _get_graph(with_qkv_bias)
    in_maps = make_in_maps(x, Wq, bq, Wk, bk, Wv, bv, Wo, with_qkv_bias)

    trace = os.environ.get("BASS_KERNEL_TRACE", "0") == "1"
    tdir = os.environ.get("BASS_KERNEL_TRACE_DIR") or None
    res = run_bass_kernel_spmd(
        nc, in_maps, list(range(NCORES)), trace=trace, tmpdir=tdir
    )
    last_exec_time_ns = res.exec_time_ns
    last_results = res

    out = np.zeros((B, S, HID), np.float32)
    for c in range(NCORES):
        out[c // GROUPS] += res.results[c]["out"]
    out += bo
    return out


# revision 46
# speedup vs baseline: 1.0215x; 1.0038x over previous
"""Multi-head attention (B=2, S=2048, H=1024, 16 heads) on 8 TRN2 NeuronCores.

Sharding: core c -> batch b = c//4, head-group g = c%4 (heads 4g..4g+3).
Each core computes q/k/v projections for its 4 heads (tensor parallel),
full attention for those heads, and a partial output projection
(contribution of its 256 hidden dims). Host sums the 4 partials per batch
and adds the output bias.

Changes over the first working kernel (268.8us -> ~219.5us):
  * software-pipelined attention inner loop: PV(k-2) is emitted two
    iterations behind scores(k), so the PE never waits on the ACT exp
    (the v1 loop serialized scores->exp->PV every jj, ~0.5us/iter stall).
  * merged scores stationary: one [128,128] kt chunk serves both heads of
    a pair; the per-head q operand is zero-padded on the other 64
    partitions ("qt_pad"), so the second LDWEIGHTS is redundant.
  * post-compile LDWEIGHTS dedupe: bass emits one LDWEIGHTS per matmul
    (~100ns of PE time each, 848 total = 85us in v1); a module pass
    removes loads whose weights AP + tile config match the immediately
    preceding load on the PE queue (only matmuls/event-sems between, no
    waits/updates on the removed load).
  * emission order maximizes consecutive same-stationary matmuls
    (qk-proj streams s-blocks per weight chunk, out-proj streams both
    e-blocks per ctxn chunk, bias/bc pairs share their stationary).
  * "big" input tiles ([128, 8, free]) loaded with ONE DMA per logical
    block via a (e p) c -> p e c rearrange: per-DMA dispatch is ~0.6us
    of queue time, so the old 26-DMA startup serialized ~15us of input;
    the pair-0 k/q weight halves go on the scalar-engine DGE queue, the
    bulk on the sync queue, cutting first-exp from ~37us to ~23us.
  * pair-1 projections and v-projections hooked INTO attention(0) so the
    exp pipeline never drains between head-pairs; partial output
    projection stores bf16 (host sums in fp32).
  * block-tail carry: each block's last two PVs + epilogue are emitted in
    the NEXT block's first two iterations, and the deferred normalize
    (early, k==4) / out-proj (boundary) split keeps the ctxn shift DMA
    off the out-proj critical path - the PE no longer parks ~1-2us at
    each of the 8 block boundaries.
  * DMA-gated hooks are emitted just-in-time (v[jj] right before its PV
    at iteration jj+2): the tile scheduler's DMA model is ~3x optimistic,
    so an early-emitted hook whose input hasn't landed blocks the
    statically-ordered PE queue head.  Queue split tuned so wv rides
    early on sync (v-projections are the first hook consumers) and
    xt sb1 on scalar behind the k/q weight halves.

Rejected experiments (measured): fp8 DoubleRow matmuls (4x PV / 2x proj
throughput) fail the 2e-2 accuracy gate (e4m3 ~2.6%/elem); custom-DVE
exp/reciprocal tables don't load under this runtime; out-DMA from PSUM
is unsupported; scalar-queue DMA dispatch during the exp phase delays
ACT; boundary-filler proj hooks serialize on the mip psum pool.
"""

import os
from contextlib import ExitStack

import numpy as np
import ml_dtypes

B = 2
S = 2048
HID = 1024
NHEAD = 16
HDIM = 64
NCORES = 8
GROUPS = 4  # head-groups per batch (cores per batch)
DH = 256  # hidden dims per core (4 heads x 64)
SCALE = 1.0 / np.sqrt(np.float32(HDIM))  # 0.125

_CACHE = {}
last_exec_time_ns = None
last_results = None


def _dedupe_ldweights(nc):
    """Remove InstLdweights that reload the PE array with the exact weights
    the previous load already placed there. bass always emits one LDWEIGHTS
    per matmul; when consecutive matmuls share lhsT the reload is ~100ns of
    pure PE overhead. Only loads with no waits/updates are removed, and the
    match window is broken by anything on the PE queue other than matmuls
    and event-semaphore instructions (and at block boundaries)."""
    import concourse.mybir as mybir

    removed = 0
    for fn in nc.m.functions:
        for bb in fn.blocks:
            insts = list(bb.instructions)
            last_sig = None
            keep = []
            for inst in insts:
                if inst.engine != mybir.EngineType.PE:
                    keep.append(inst)
                    continue
                tname = type(inst).__name__
                if tname == "InstLdweights":
                    sig = (
                        str(inst.ins[0]),
                        str(inst.perf_mode),
                        str(inst.is_transpose),
                        str(inst.tile_position),
                        str(inst.tile_size),
                    )
                    if (
                        sig == last_sig
                        and not inst.has_wait()
                        and not inst.has_update()
                    ):
                        removed += 1
                        continue  # drop it
                    last_sig = sig
                    keep.append(inst)
                elif tname == "InstMatmult":
                    keep.append(inst)
                elif tname == "InstEventSemaphore":
                    keep.append(inst)
                else:
                    # drain/branch/anything else: invalidate the window
                    last_sig = None
                    keep.append(inst)
            if removed:
                bb.instructions = keep
    return removed


def _build_graph(with_qkv_bias: bool):
    import concourse.bass as bass
    import concourse.mybir as mybir
    import concourse.tile as tile
    from concourse import bacc

    F32 = mybir.dt.float32
    BF16 = mybir.dt.bfloat16
    EXP = mybir.ActivationFunctionType.Exp
    LN = mybir.ActivationFunctionType.Ln

    # Keep the activation-table steering from v1: with it, the act-table
    # load pass settles on one table set (1 load) instead of 17.
    if not getattr(bacc, "_mha_act_tabs_patched", False):
        orig_gat = bacc.get_activation_tables

        def _gat(arch, _orig=orig_gat):
            out = {}
            for n, s in _orig(arch).items():
                if n != "natural_log_exp_and_others":
                    s = s - {EXP, LN}
                out[n] = s
            return out

        bacc.get_activation_tables = _gat
        bacc._mha_act_tabs_patched = True

    nc = bacc.Bacc()
    xt_d = nc.declare_dram_parameter("xt", [HID, S], BF16, isOutput=False)
    wq_d = nc.declare_dram_parameter("wq", [HID, DH], BF16, isOutput=False)
    wk_d = nc.declare_dram_parameter("wk", [HID, DH], BF16, isOutput=False)
    wv_d = nc.declare_dram_parameter("wv", [HID, DH], BF16, isOutput=False)
    wo_d = nc.declare_dram_parameter("wo", [DH, HID], BF16, isOutput=False)
    if with_qkv_bias:
        bq_d = nc.declare_dram_parameter("bq", [1, DH], BF16, isOutput=False)
        bk_d = nc.declare_dram_parameter("bk", [1, DH], BF16, isOutput=False)
        bv_d = nc.declare_dram_parameter("bv", [1, DH], BF16, isOutput=False)
    out_d = nc.declare_dram_parameter("out", [S, HID], BF16, isOutput=True)

    with ExitStack() as ctx:
        tc = ctx.enter_context(tile.TileContext(nc))
        cons = ctx.enter_context(tc.tile_pool(name="cons", bufs=1))
        work = ctx.enter_context(tc.tile_pool(name="work", bufs=3))
        scp = ctx.enter_context(tc.tile_pool(name="scp", bufs=2, space="PSUM"))
        pvp = ctx.enter_context(tc.tile_pool(name="pvp", bufs=1, space="PSUM"))
        mip = ctx.enter_context(tc.tile_pool(name="mip", bufs=2, space="PSUM"))

        # ---- persistent SBUF tiles ------------------------------------
        # "big" tiles pack all 8 hidden chunks: [128, 8, free]. One DMA
        # moves a whole logical block (dispatch cost was serializing the
        # startup at ~0.6us per DMA on the queues).
        xt_big = cons.tile([128, 8, S], BF16, name="xt", tag="xt")
        wq_big = cons.tile([128, 8, DH], BF16, name="wq", tag="wq")
        wk_big = cons.tile([128, 8, DH], BF16, name="wk", tag="wk")
        wv_big = cons.tile([128, 8, DH], BF16, name="wv", tag="wv")
        xt_sb = [xt_big[:, e, :] for e in range(8)]
        wq_sb = [wq_big[:, e, :] for e in range(8)]
        wk_sb = [wk_big[:, e, :] for e in range(8)]
        wv_sb = [wv_big[:, e, :] for e in range(8)]
        wo_sb = [
            cons.tile([128, HID], BF16, name=f"wo{e}", tag=f"wo{e}") for e in range(2)
        ]

        def load_big(eng, out_ap, dram, cols):
            eng.dma_start(
                out=out_ap,
                in_=dram[:, cols].rearrange("(e p) c -> p e c", e=8),
            )

        # scalar queue: the small pair-0 weight halves that gate the first
        # k/q chains, then the last xt block.  sync queue: everything else
        # in consumption order.
        # scalar queue: pair-0 k/q weight halves (gate the first chains),
        # then xt sb1 and the pair-1 halves.  sync queue: xt sb0 halves,
        # wv early (v-projections start at ~iteration 0), then the rest.
        load_big(nc.scalar, wk_big[:, :, 0:128], wk_d, slice(0, 128))
        load_big(nc.scalar, wq_big[:, :, 0:128], wq_d, slice(0, 128))
        for half in range(2):
            rows = slice(half * 512, (half + 1) * 512)
            nc.scalar.dma_start(
                out=xt_big[:, half * 4 : (half + 1) * 4, 512:1024],
                in_=xt_d[rows, 512:1024].rearrange("(e p) c -> p e c", e=4),
            )
        load_big(nc.scalar, wk_big[:, :, 128:256], wk_d, slice(128, 256))
        load_big(nc.scalar, wq_big[:, :, 128:256], wq_d, slice(128, 256))
        for half in range(2):
            rows = slice(half * 512, (half + 1) * 512)
            nc.sync.dma_start(
                out=xt_big[:, half * 4 : (half + 1) * 4, 0:512],
                in_=xt_d[rows, 0:512].rearrange("(e p) c -> p e c", e=4),
            )
        load_big(nc.sync, wv_big, wv_d, slice(0, DH))
        load_big(nc.sync, xt_big[:, :, 1024:1536], xt_d, slice(1024, 1536))
        load_big(nc.sync, xt_big[:, :, 1536:2048], xt_d, slice(1536, 2048))
        for e in range(2):
            nc.sync.dma_start(out=wo_sb[e], in_=wo_d[e * 128 : (e + 1) * 128, :])

        ones1 = cons.tile([1, 512], BF16, name="ones1", tag="ones1")
        nc.vector.memset(ones1, 1.0)
        # stationary of the K=1 broadcast matmul for 1/l (partition base 0)
        ones1b = cons.tile([1, 64], BF16, name="ones1b", tag="ones1b")
        nc.vector.memset(ones1b, 1.0)

        if with_qkv_bias:
            bias_sb = {}
            for nm, d in (("bq", bq_d), ("bk", bk_d), ("bv", bv_d)):
                t = cons.tile([1, DH], BF16, name=f"{nm}s", tag=f"{nm}s")
                nc.sync.dma_start(out=t, in_=d)
                bias_sb[nm] = t

        # qt zero-padded per (pair, h): data rows h*64..h*64+63, rest 0.
        qt_pad = [
            [
                cons.tile([128, S], BF16, name=f"qp{c}{h}", tag=f"qp{c}{h}")
                for h in range(2)
            ]
            for c in range(2)
        ]
        # zero them once (proj only overwrites the data half); gpsimd for
        # half of them to spread the startup memset cost.
        nc.vector.memset(qt_pad[0][0][64:128, :], 0.0)
        nc.vector.memset(qt_pad[0][1][0:64, :], 0.0)
        nc.gpsimd.memset(qt_pad[1][0][64:128, :], 0.0)
        nc.gpsimd.memset(qt_pad[1][1][0:64, :], 0.0)

        kt_sb = [
            cons.tile([128, S], BF16, name=f"kt{c}", tag=f"kt{c}") for c in range(2)
        ]
        v_sb = [
            cons.tile([128, 4, 65], BF16, name=f"v{j}", tag=f"v{j}") for j in range(16)
        ]
        ctxn_sb = [
            [
                cons.tile([128, 512], BF16, name=f"cx{c}_{i}", tag=f"cx{c}_{i}")
                for i in range(4)
            ]
            for c in range(2)
        ]

        # ---- projections ----------------------------------------------
        def proj_qk(which, pair, sbs):
            """q or k projection for one pair over the given s-blocks,
            streaming all of them per weight chunk so consecutive matmuls
            share their stationary (dedupable LDWEIGHTS)."""
            w_sb = wq_sb if which == "q" else wk_sb
            ps = [
                mip.tile([128, 512], F32, name=f"p{which}{pair}{sb}", tag="mm")
                for sb in sbs
            ]
            for e in range(8):
                lhs = w_sb[e][:, pair * 128 : (pair + 1) * 128]
                for i, sb in enumerate(sbs):
                    nc.tensor.matmul(
                        ps[i],
                        lhsT=lhs,
                        rhs=xt_sb[e][:, sb * 512 : (sb + 1) * 512],
                        start=(e == 0),
                        stop=(e == 7 and not with_qkv_bias),
                    )
            if with_qkv_bias:
                bias = bias_sb["bq" if which == "q" else "bk"]
                lhs = bias[:, pair * 128 : (pair + 1) * 128]
                for i in range(len(sbs)):
                    nc.tensor.matmul(ps[i], lhsT=lhs, rhs=ones1, start=False, stop=True)
            for i, sb in enumerate(sbs):
                sl = slice(sb * 512, (sb + 1) * 512)
                if which == "k":
                    nc.vector.tensor_copy(out=kt_sb[pair][:, sl], in_=ps[i])
                else:
                    nc.vector.tensor_copy(
                        out=qt_pad[pair][0][0:64, sl], in_=ps[i][0:64, :]
                    )
                    nc.vector.tensor_copy(
                        out=qt_pad[pair][1][64:128, sl], in_=ps[i][64:128, :]
                    )

        def proj_v_one(jj):
            # v [s, d] natural, per j-chunk [128, 4, 65] with ones at 64.
            ps = mip.tile([128, DH], F32, name=f"pv{jj}", tag="mm")
            for e in range(8):
                nc.tensor.matmul(
                    ps,
                    lhsT=xt_sb[e][:, jj * 128 : (jj + 1) * 128],
                    rhs=wv_sb[e],
                    start=(e == 0),
                    stop=(e == 7 and not with_qkv_bias),
                )
            if with_qkv_bias:
                nc.tensor.matmul(
                    ps,
                    lhsT=ones1[:, 0:128],
                    rhs=bias_sb["bv"],
                    start=False,
                    stop=True,
                )
            nc.vector.tensor_copy(
                out=v_sb[jj][:, :, 0:64],
                in_=ps.rearrange("p (h d) -> p h d", h=4),
            )
            nc.vector.memset(v_sb[jj][:, :, 64:65], 1.0)

        # ---- deferred epilogue work ------------------------------------
        # part2 (bc matmuls + normalize + ctxn shift DMA) flushes EARLY in
        # the following block so the shift DMA is long done when the
        # out-projection (flushed at that block's END) reads ctxn rows
        # 64-127 — otherwise the PE parks ~1us on the DMA at the boundary.
        deferred_early = []
        deferred_late = []

        def flush_deferred(lst):
            while lst:
                lst.pop(0)()

        def outproj(ib):
            # partial output projection, e-block-streamed per ctxn chunk,
            # PSUM DMA'd straight to DRAM
            for ss in range(4):
                po = [
                    mip.tile([128, 512], F32, name=f"po{ib}{ss}{i}", tag="mm")
                    for i in range(2)
                ]
                for cc in range(2):
                    lhs = ctxn_sb[cc][ib][:, ss * 128 : (ss + 1) * 128]
                    for eb in range(2):
                        nc.tensor.matmul(
                            po[eb],
                            lhsT=lhs,
                            rhs=wo_sb[cc][:, eb * 512 : (eb + 1) * 512],
                            start=(cc == 0),
                            stop=(cc == 1),
                        )
                row = ib * 512 + ss * 128
                for eb in range(2):
                    ot = work.tile(
                        [128, 512], BF16, name=f"ot{ib}{ss}{eb}", tag="ot", bufs=4
                    )
                    nc.vector.tensor_copy(out=ot, in_=po[eb])
                    # the final block's stores race the kernel end; spread
                    # them over both DGE queues (ACT is idle by then)
                    eng = nc.scalar if (ib == 3 and eb == 1) else nc.sync
                    eng.dma_start(
                        out=out_d[row : row + 128, eb * 512 : (eb + 1) * 512],
                        in_=ot,
                    )

        # ---- attention -------------------------------------------------
        # Each block's last two PVs (which wait on the freshest exps) and
        # its epilogue are carried into the NEXT block's first iterations,
        # so the PE never parks at a block boundary.
        pending = []

        def epilogue(pair, ib, pv):
            # psum->sbuf copies free the pv banks; 1/l as exp(-ln(l)) on
            # ACT (custom-DVE reciprocal tables don't load in this
            # environment's compiler path).
            pvs_l, rl16_l = [], []
            for h in range(2):
                pvs = work.tile(
                    [65, 512], F32, name=f"pvs{pair}{ib}{h}", tag="pvs", bufs=6
                )
                nc.vector.tensor_copy(out=pvs, in_=pv[h])
                lnl = work.tile(
                    [1, 512], F32, name=f"lnl{pair}{ib}{h}", tag="lnl", bufs=2
                )
                nc.scalar.activation(out=lnl, in_=pvs[64:65, :], func=LN)
                rl16 = work.tile(
                    [1, 512], BF16, name=f"rl16{pair}{ib}{h}", tag="rl16", bufs=6
                )
                nc.scalar.activation(out=rl16, in_=lnl, func=EXP, scale=-1.0)
                pvs_l.append(pvs)
                rl16_l.append(rl16)

            def part2(pair=pair, ib=ib, pvs_l=pvs_l, rl16_l=rl16_l):
                bc_t = []
                for h in range(2):
                    bc = mip.tile([64, 512], F32, name=f"bc{pair}{ib}{h}", tag="mm")
                    nc.tensor.matmul(
                        bc, lhsT=ones1b, rhs=rl16_l[h], start=True, stop=True
                    )
                    bc_t.append(bc)
                # DVE may read only one PSUM operand: in0 SBUF, in1 PSUM.
                nc.vector.tensor_mul(
                    out=ctxn_sb[pair][ib][0:64, :],
                    in0=pvs_l[0][0:64, :],
                    in1=bc_t[0],
                )
                tmp = work.tile([64, 512], BF16, name=f"tmp{pair}{ib}", tag="tmp")
                nc.vector.tensor_mul(out=tmp, in0=pvs_l[1][0:64, :], in1=bc_t[1])
                nc.sync.dma_start(out=ctxn_sb[pair][ib][64:128, :], in_=tmp)

            deferred_early.append(part2)
            if pair == 1:
                deferred_late.append(lambda ib=ib: outproj(ib))

        def attention(pair, hooks=None, last=False):
            hooks = hooks or {}
            for ib in range(4):
                pv = [
                    pvp.tile([65, 512], F32, name=f"pva{pair}{ib}", tag="pva"),
                    pvp.tile([65, 512], F32, name=f"pvb{pair}{ib}", tag="pvb"),
                ]
                ex_t = [None] * 16

                def emit_pv(jj, pair=pair, pv=pv, ex_t=ex_t):
                    for h in range(2):
                        nc.tensor.matmul(
                            pv[h],
                            lhsT=v_sb[jj][:, pair * 2 + h, :],
                            rhs=ex_t[jj][:, h * 512 : (h + 1) * 512],
                            start=(jj == 0),
                            stop=(jj == 15),
                        )

                for k in list(range(16)) + [16]:
                    for fn in hooks.get((ib, k), ()):
                        fn()
                    if k == 16:
                        break
                    ps = scp.tile([128, 1024], F32, name=f"sc{pair}{ib}{k}", tag="sc")
                    ktsl = kt_sb[pair][:, k * 128 : (k + 1) * 128]
                    for h in range(2):
                        nc.tensor.matmul(
                            ps[:, h * 512 : (h + 1) * 512],
                            lhsT=ktsl,
                            rhs=qt_pad[pair][h][:, ib * 512 : (ib + 1) * 512],
                            start=True,
                            stop=True,
                        )
                    ex = work.tile(
                        [128, 1024], BF16, name=f"ex{pair}{ib}{k}", tag="ex", bufs=4
                    )
                    nc.scalar.activation(out=ex, in_=ps, func=EXP, scale=float(SCALE))
                    ex_t[k] = ex
                    if k == 0 and pending:
                        pending[0]()
                    elif k == 1 and pending:
                        pending[1]()
                        pending.clear()
                    elif k == 7:
                        # late enough that the previous block's LN/EXP
                        # reciprocal chain has drained through ACT (the bc
                        # matmuls otherwise park the PE ~2us waiting on it)
                        flush_deferred(deferred_early)
                    if k >= 2:
                        emit_pv(k - 2)

                # out-proj of the PREVIOUS block lands at the boundary (its
                # ctxn was finished early in this block); the last block
                # leaves it for the final tail so the fresh PVs go first.
                if not (last and ib == 3):
                    flush_deferred(deferred_late)
                pending.append(lambda e=emit_pv: e(14))
                pending.append(
                    lambda e=emit_pv, p=pair, i=ib, t=pv: (e(15), epilogue(p, i, t))
                )

        # ---- emission order --------------------------------------------
        # minimal prelude: the first scores chunk only needs kt sb0 and
        # qt sb0, each an 8-matmul chain off the first DMA wave.
        proj_qk("k", 0, [0])
        proj_qk("q", 0, [0])

        hooks0 = {}
        # remaining pair-0 projections, j-chunk v projections, and ALL of
        # pair-1's projections are hooked into attention(0) so the exp
        # pipeline never drains.  DMA-gated work is hooked just-in-time
        # (v[jj] right before its PV at iteration jj+2) — an early hook
        # whose input hasn't landed blocks the statically-ordered PE
        # queue head and stalls everything behind it.
        for jj in range(14):
            hooks0[(0, jj + 2)] = [lambda jj=jj: proj_v_one(jj)]
        hooks0[(0, 16)] = [lambda: proj_v_one(14), lambda: proj_v_one(15)]
        hooks0[(0, 3)].append(lambda: proj_qk("k", 0, [1]))
        hooks0[(0, 6)].append(lambda: proj_qk("k", 0, [2, 3]))
        hooks0[(0, 10)].append(lambda: proj_qk("q", 0, [1]))
        hooks0[(1, 2)] = [lambda: proj_qk("q", 0, [2, 3])]
        hooks0[(2, 2)] = [lambda: proj_qk("k", 1, [0, 1])]
        hooks0[(2, 9)] = [lambda: proj_qk("k", 1, [2, 3])]
        hooks0[(3, 2)] = [lambda: proj_qk("q", 1, [0, 1])]
        hooks0[(3, 9)] = [lambda: proj_qk("q", 1, [2, 3])]

        attention(0, hooks0)
        attention(1, last=True)
        # final tail: the last block's PVs + epilogue go FIRST (their exps
        # are the freshest dependency); the held-back out-proj of block 6
        # fills the epilogue's ACT latency; then normalize + out-proj 7.
        for fn in pending:
            fn()
        pending.clear()
        if deferred_late:
            deferred_late.pop(0)()  # out-proj of the second-to-last block
        flush_deferred(deferred_early)
        flush_deferred(deferred_late)

    nc.compile()
    n = _dedupe_ldweights(nc)
    nc._mha_deduped_ldweights = n
    return nc


def _get_graph(with_qkv_bias: bool):
    key = ("nc", with_qkv_bias)
    if key not in _CACHE:
        _CACHE[key] = _build_graph(with_qkv_bias)
    return _CACHE[key]


def make_in_maps(x, Wq, bq, Wk, bk, Wv, bv, Wo, with_qkv_bias):
    bf16 = ml_dtypes.bfloat16
    in_maps = []
    for c in range(NCORES):
        b, g = c // GROUPS, c % GROUPS
        hs = slice(g * DH, (g + 1) * DH)
        m = {
            "xt": np.ascontiguousarray(x[b].T.astype(bf16)),
            "wq": np.ascontiguousarray(Wq[hs, :].T.astype(bf16)),
            "wk": np.ascontiguousarray(Wk[hs, :].T.astype(bf16)),
            "wv": np.ascontiguousarray(Wv[hs, :].T.astype(bf16)),
            "wo": np.ascontiguousarray(Wo[:, hs].T.astype(bf16)),
        }
        if with_qkv_bias:
            m["bq"] = np.ascontiguousarray(bq[None, hs].astype(bf16))
            m["bk"] = np.ascontiguousarray(bk[None, hs].astype(bf16))
            m["bv"] = np.ascontiguousarray(bv[None, hs].astype(bf16))
        in_maps.append(m)
    return in_maps


def kernel(x, Wq, bq, Wk, bk, Wv, bv, Wo, bo):
    global last_exec_time_ns, last_results
    from concourse.bass_utils import run_bass_kernel_spmd

    x = np.asarray(x, np.float32)
    Wq = np.asarray(Wq, np.float32)
    Wk = np.asarray(Wk, np.float32)
    Wv = np.asarray(Wv, np.float32)
    Wo = np.asarray(Wo, np.float32)
    bq = np.asarray(bq, np.float32)
    bk = np.asarray(bk, np.float32)
    bv = np.asarray(bv, np.float32)
    bo = np.asarray(bo, np.float32)

    with_qkv_bias = bool(np.any(bq) or np.any(bk) or np.any(bv))
    nc = _get_graph(with_qkv_bias)
    in_maps = make_in_maps(x, Wq, bq, Wk, bk, Wv, bv, Wo, with_qkv_bias)

    trace = os.environ.get("BASS_KERNEL_TRACE", "0") == "1"
    tdir = os.environ.get("BASS_KERNEL_TRACE_DIR") or None
    res = run_bass_kernel_spmd(
        nc, in_maps, list(range(NCORES)), trace=trace, tmpdir=tdir
    )
    last_exec_time_ns = res.exec_time_ns
    last_results = res

    out = np.zeros((B, S, HID), np.float32)
    for c in range(NCORES):
        out[c // GROUPS] += res.results[c]["out"]
    out += bo
    return out
